# revision 29
# baseline (speedup 1.0000x reference)
import functools
import sys

import numpy as np

sys.path.insert(0, "/opt/trn_rl_repo")

import concourse.bass as bass
import concourse.bacc as bacc
import concourse.mybir as mybir
from concourse import tile

B, L, D = 256, 256, 64
NCORES = 8
BP = B // NCORES
LM = L - 1
N2 = 2 * LM
NBLK = 17
BLK = LM // NBLK
XF = BP * 2 * D
CROWS = BP * L + BP * L // 8
TROWS = CROWS + 8
F32 = mybir.dt.float32
F16 = mybir.dt.float16
I16 = mybir.dt.int16
U8 = mybir.dt.uint8
ADD = mybir.AluOpType.add
MULT = mybir.AluOpType.mult
EQ = mybir.AluOpType.is_equal
AND = mybir.AluOpType.bitwise_and
SHR = mybir.AluOpType.logical_shift_right
COPY = mybir.ActivationFunctionType.Copy


def _build_program():
    nc = bacc.Bacc(None, target_bir_lowering=False)
    xin_d = nc.declare_dram_parameter("xin", [TROWS, D], U8, isOutput=False)
    yin_d = nc.declare_dram_parameter("yin", [TROWS, D], U8, isOutput=False)
    out_d = nc.declare_dram_parameter("out", [BP, 1], F32, isOutput=True)
    A_d = nc.dram_tensor("A_scratch", [BP, LM, LM], F32)

    with tile.TileContext(nc) as tc:
        with (
            tc.tile_pool(name="const", bufs=1) as cpool,
            tc.tile_pool(name="ps", bufs=2, space="PSUM") as pspool,
            tc.tile_pool(name="ev", bufs=3) as evpool,
            tc.tile_pool(name="pde", bufs=1) as upool,
            tc.tile_pool(name="ablk", bufs=2) as apool,
            tc.tile_pool(name="tmp", bufs=2) as tpool,
        ):
            def load_unpack(in_d, tag):
                lo_d = in_d[0 : BP * L, :]
                hp_d = in_d[BP * L : CROWS, :]
                scl = cpool.tile([128, 4], U8, name=f"scl_{tag}")
                nc.gpsimd.dma_start(
                    out=scl[:],
                    in_=in_d[CROWS:TROWS, :].rearrange(
                        "a (b c) -> (a b) c", b=16, c=4
                    ),
                )
                loq = cpool.tile([128, XF], U8, name=f"loq_{tag}")
                hpq = cpool.tile([128, XF // 8], U8, name=f"hpq_{tag}")
                nc.gpsimd.dma_start(
                    out=loq[:].rearrange("q (p c d) -> q p c d", p=BP, c=2),
                    in_=lo_d.rearrange("(p c q) d -> q p c d", p=BP, c=2),
                )
                nc.gpsimd.dma_start(
                    out=hpq[:].rearrange("q (p c d) -> q p c d", p=BP // 8, c=2),
                    in_=hp_d.rearrange("(p c q) d -> q p c d", p=BP // 8, c=2),
                )
                Q = XF // 8
                hf = cpool.tile([128, XF], U8, name=f"hf_{tag}")
                sh = cpool.tile([128, Q], U8, name=f"sh_{tag}")
                nc.vector.tensor_scalar(hf[:, 0:Q], hpq[:], 1, None, AND)
                for j in range(1, 7):
                    nc.vector.tensor_scalar(sh[:], hpq[:], j, None, SHR)
                    nc.vector.tensor_scalar(
                        hf[:, j * Q : (j + 1) * Q], sh[:], 1, None, AND
                    )
                nc.vector.tensor_scalar(hf[:, 7 * Q : 8 * Q], hpq[:], 7, None, SHR)
                ht = cpool.tile([128, XF], I16, name=f"ht_{tag}")
                nc.vector.tensor_scalar(ht[:], hf[:], 256, -256, MULT, ADD)
                xb = cpool.tile([128, XF], I16, name=f"xb_{tag}")
                nc.vector.tensor_tensor(xb[:], ht[:], loq[:], ADD)
                xq = cpool.tile([128, XF], F16, name=f"xq_{tag}")
                nc.vector.tensor_scalar(
                    xq[:], xb[:], scl[:].bitcast(F32), None, MULT,
                )
                return xq

            xq = load_unpack(xin_d, "x")
            yq = load_unpack(yin_d, "y")

            def x_ap(p, c):
                o = (p * 2 + c) * D
                return xq[:, o : o + D]

            def y_ap(p, c):
                o = (p * 2 + c) * D
                return yq[:, o : o + D]

            iot = cpool.tile([128, 2 * LM], I16)
            m1 = cpool.tile([128, 2 * LM], F16)
            m0 = cpool.tile([128, 2 * LM], F16)
            dTq = cpool.tile([128, 2 * LM], F16)
            nc.gpsimd.iota(iot[:], [[128, 2], [-1, LM]], base=0, channel_multiplier=1)
            nc.vector.tensor_scalar(m1[:], iot[:], 1, None, EQ)
            nc.vector.tensor_scalar(m0[:], iot[:], 0, None, EQ)
            nc.vector.tensor_sub(dTq[:], m1[:], m0[:])

            def dT_ap(c):
                return dTq[:, c * LM : (c + 1) * LM]

            for p in range(BP):
                dxT_ps = pspool.tile([D, LM], F32, tag="dxps", name="dxT_ps")
                dyT_ps = pspool.tile([D, LM], F32, tag="dyps", name="dyT_ps")
                for c in range(2):
                    nc.tensor.matmul(
                        dxT_ps[:], x_ap(p, c), dT_ap(c),
                        start=(c == 0), stop=(c == 1),
                    )
                for c in range(2):
                    nc.tensor.matmul(
                        dyT_ps[:], y_ap(p, c), dT_ap(c),
                        start=(c == 0), stop=(c == 1),
                    )
                dxT_sb = evpool.tile([D, LM], F32, tag="dxe", name="dxT_sb")
                dyT_sb = evpool.tile([D, LM], F32, tag="dye", name="dyT_sb")
                nc.scalar.activation(dxT_sb[:], dxT_ps[:], COPY, scale=0.5)
                nc.scalar.activation(dyT_sb[:], dyT_ps[:], COPY, scale=0.5)
                for m0_, m1_ in ((0, 128), (128, LM)):
                    a_ps = pspool.tile([128, LM], F32, tag="aps", name="a_ps")
                    nc.tensor.matmul(
                        a_ps[: m1_ - m0_, :], dxT_sb[:, m0_:m1_], dyT_sb[:],
                        start=True, stop=True,
                    )
                    a_sb = evpool.tile([128, LM], F32, tag="aev", name="a_sb", bufs=64)
                    nc.scalar.activation(
                        a_sb[: m1_ - m0_, :], a_ps[: m1_ - m0_, :], COPY, bias=-1.0
                    )
                    nc.sync.dma_start(out=A_d[p][m0_:m1_, :], in_=a_sb[: m1_ - m0_, :])

            u_bufs = [
                upool.tile([BP, N2 + 1], F32, tag=f"u{i}", name=f"u{i}")
                for i in range(2)
            ]
            nc.vector.memset(u_bufs[0][:], 1.0)
            nc.vector.memset(u_bufs[1][:], 1.0)
            step = 0
            for b in range(NBLK):
                ablk = apool.tile([BP, BLK * LM], F32, tag="ablk", name="ablk")
                nc.sync.dma_start(
                    out=ablk[:],
                    in_=A_d[:, b * BLK : (b + 1) * BLK, :].rearrange(
                        "p r a -> p (r a)"
                    ),
                )
                for r in range(BLK):
                    base = ablk[:, r * LM : (r + 1) * LM]
                    dbl = bass.AP(
                        base.tensor,
                        base.offset,
                        [base.ap[0], [base.ap[1][0], LM], [0, 2]],
                    )
                    for _ in range(2):
                        up = u_bufs[step % 2]
                        un = u_bufs[(step + 1) % 2]
                        tmp = tpool.tile([BP, N2], F32, tag="tmp", name="tmp")
                        nc.gpsimd.tensor_mul(tmp[:], up[:, 0:N2], dbl)
                        nc.vector.tensor_tensor_scan(
                            un[:, 1 : N2 + 1], up[:, 1 : N2 + 1], tmp[:],
                            1.0, ADD, ADD,
                        )
                        step += 1
            nc.sync.dma_start(out=out_d[:], in_=u_bufs[step % 2][:, N2 : N2 + 1])
    nc.compile()
    return nc


@functools.lru_cache(maxsize=1)
def _program():
    return _build_program()


@functools.lru_cache(maxsize=1)
def _executor():
    import jax
    from jax.sharding import Mesh, PartitionSpec
    from jax.experimental.shard_map import shard_map
    from concourse import bass2jax
    from concourse.bass2jax import _bass_exec_p, install_neuronx_cc_hook

    nc = _program()
    install_neuronx_cc_hook()
    partition_name = (
        nc.partition_id_tensor.name if nc.partition_id_tensor is not None else None
    )
    in_names: list[str] = []
    out_names: list[str] = []
    out_avals = []
    zero_specs = []
    for alloc in nc.m.functions[0].allocations:
        if not isinstance(alloc, mybir.MemoryLocationSet):
            continue
        name = alloc.memorylocations[0].name
        if alloc.kind == "ExternalInput":
            if name != partition_name:
                in_names.append(name)
        elif alloc.kind == "ExternalOutput":
            shape = tuple(alloc.tensor_shape)
            dtype = mybir.dt.np(alloc.dtype)
            out_names.append(name)
            out_avals.append(jax.core.ShapedArray(shape, dtype))
            zero_specs.append((shape, dtype))
    n_params = len(in_names)
    n_outs = len(out_avals)
    in_names_all = in_names + out_names + (
        [partition_name] if partition_name else []
    )
    donate = tuple(range(n_params, n_params + n_outs))

    def _body(*args):
        operands = list(args)
        if partition_name is not None:
            operands.append(bass2jax.partition_id_tensor())
        outs = _bass_exec_p.bind(
            *operands,
            out_avals=tuple(out_avals),
            in_names=tuple(in_names_all),
            out_names=tuple(out_names),
            lowering_input_output_aliases=(),
            sim_require_finite=True,
            sim_require_nnan=True,
            nc=nc,
        )
        return tuple(outs)

    devices = jax.devices()[:NCORES]
    assert len(devices) == NCORES
    mesh = Mesh(np.asarray(devices), ("core",))
    in_specs = (PartitionSpec("core"),) * (n_params + n_outs)
    out_specs = (PartitionSpec("core"),) * len(out_names)
    sharded = jax.jit(
        shard_map(
            _body, mesh=mesh, in_specs=in_specs, out_specs=out_specs,
            check_rep=False,
        ),
        donate_argnums=donate,
        keep_unused=True,
    )
    return sharded, in_names, out_names, zero_specs


_C_SRC = r"""
#include <stdint.h>
#include <math.h>

float absmax_f32(const float* x, long n) {
    float m = 0.f;
    for (long i = 0; i < n; i++) {
        float v = fabsf(x[i]);
        if (v > m) m = v;
    }
    return m;
}

/* x: [ncores][32][256*64] f32 -> out: per core 32*PL lo bytes then 4*PL
   packed-hi bytes (core-major, so axis-0 sharding slices stay contiguous).
   9-bit value m = round(x*k + 256) in [0,511]; lo = m & 255; hp packs the
   hi bit of pairs (pb, pb+4, pb+8, ..., pb+28) into one byte. */
void quant9(const float* x, uint8_t* out, float k, long ncores,
            long core_stride) {
    const long PL = 256 * 64;
    for (long c = 0; c < ncores; c++) {
        const float* xc = x + c * 32 * PL;
        uint8_t* loc = out + c * core_stride;
        uint8_t* hpc = loc + 32 * PL;
        for (long pb = 0; pb < 4; pb++) {
            const float* xp[8];
            uint8_t* lp[8];
            for (int j = 0; j < 8; j++) {
                xp[j] = xc + (pb + 4 * j) * PL;
                lp[j] = loc + (pb + 4 * j) * PL;
            }
            uint8_t* h = hpc + pb * PL;
            for (long i = 0; i < PL; i++) {
                int acc = 0;
                for (int j = 0; j < 8; j++) {
                    int m = (int)(xp[j][i] * k + 256.5f);
                    m = m < 0 ? 0 : (m > 511 ? 511 : m);
                    lp[j][i] = (uint8_t)(m & 255);
                    acc |= (m >> 8) << j;
                }
                h[i] = (uint8_t)acc;
            }
        }
    }
}
"""


def _build_cquant():
    import ctypes
    import hashlib
    import os
    import subprocess

    h = hashlib.md5(_C_SRC.encode()).hexdigest()[:12]
    so = f"/tmp/_sigq_{h}.so"
    if not os.path.exists(so):
        cpath = f"/tmp/_sigq_{h}.c"
        with open(cpath, "w") as f:
            f.write(_C_SRC)
        tmp = so + f".{os.getpid()}.tmp"
        subprocess.run(
            ["gcc", "-O3", "-march=native", "-funroll-loops", "-shared",
             "-fPIC", cpath, "-o", tmp, "-lm"],
            check=True, capture_output=True,
        )
        os.replace(tmp, so)
    lib = ctypes.CDLL(so)
    lib.absmax_f32.restype = ctypes.c_float
    lib.absmax_f32.argtypes = [ctypes.c_void_p, ctypes.c_long]
    lib.quant9.restype = None
    lib.quant9.argtypes = [
        ctypes.c_void_p, ctypes.c_void_p, ctypes.c_float, ctypes.c_long,
        ctypes.c_long,
    ]
    return lib


try:
    _clib = _build_cquant()
except Exception:
    _clib = None


def _scale_rows(v: float) -> np.ndarray:
    return np.full((128,), v, np.float32).view(np.uint8).reshape(8, D)


def _quant9_c(arr: np.ndarray):
    a = np.ascontiguousarray(arr, np.float32).reshape(-1)
    s = float(_clib.absmax_f32(a.ctypes.data, a.size)) * 1.002
    if s == 0.0:
        s = 1.0
    comb = np.empty(NCORES * TROWS * D, np.uint8)
    _clib.quant9(a.ctypes.data, comb.ctypes.data,
                 np.float32(256.0 / s), NCORES, TROWS * D)
    v = comb.reshape(NCORES, TROWS, D)
    v[:, CROWS:TROWS] = _scale_rows(s / 256.0)
    return comb.reshape(NCORES * TROWS, D)


def _pack_fallback(arr: np.ndarray):
    a = np.ascontiguousarray(arr, np.float32).reshape(-1)
    s = float(np.abs(a).max()) * 1.002
    if s == 0.0:
        s = 1.0
    m = np.clip(np.rint(a * (256.0 / s)).astype(np.int32) + 256, 0, 511)
    lo = (m & 255).astype(np.uint8).reshape(NCORES, BP * L, D)
    h1 = (m >> 8).astype(np.uint8).reshape(NCORES, 8, 4, L * D)
    hp = np.zeros((NCORES, 4, L * D), np.uint8)
    for j in range(8):
        hp |= (h1[:, j] << j).astype(np.uint8)
    comb = np.empty((NCORES, TROWS, D), np.uint8)
    comb[:, 0 : BP * L] = lo
    comb[:, BP * L : CROWS] = hp.reshape(NCORES, BP * L // 8, D)
    comb[:, CROWS:TROWS] = _scale_rows(s / 256.0)
    return comb.reshape(NCORES * TROWS, D)


@functools.lru_cache(maxsize=1)
def _sharding():
    import jax
    from jax.sharding import Mesh, NamedSharding, PartitionSpec

    mesh = Mesh(np.asarray(jax.devices()[:NCORES]), ("core",))
    return NamedSharding(mesh, PartitionSpec("core"))


def kernel(xs: np.ndarray, ys: np.ndarray) -> np.ndarray:
    import jax

    sharded, in_names, out_names, zero_specs = _executor()
    sh = _sharding()
    quant = _quant9_c if _clib is not None else _pack_fallback
    feeds = {"xin": jax.device_put(quant(np.asarray(xs)), sh)}
    feeds["yin"] = jax.device_put(quant(np.asarray(ys)), sh)
    concat_in = [feeds[name] for name in in_names]
    concat_zeros = [
        np.zeros((NCORES * s[0], *s[1:]), dt) for s, dt in zero_specs
    ]
    out_arrs = sharded(*concat_in, *concat_zeros)
    out = np.asarray(out_arrs[out_names.index("out")])
    return out.reshape(B).astype(np.float32, copy=False)


# revision 30
# speedup vs baseline: 1.0367x; 1.0367x over previous
import functools
import sys

import numpy as np

sys.path.insert(0, "/opt/trn_rl_repo")

import concourse.bass as bass
import concourse.bacc as bacc
import concourse.mybir as mybir
from concourse import tile

B, L, D = 256, 256, 64
NCORES = 8
BP = B // NCORES
LM = L - 1
N2 = 2 * LM
NBLK = 17
BLK = LM // NBLK
XF = BP * 2 * D
F32 = mybir.dt.float32
F16 = mybir.dt.float16
I16 = mybir.dt.int16
U8 = mybir.dt.uint8
ADD = mybir.AluOpType.add
MULT = mybir.AluOpType.mult
EQ = mybir.AluOpType.is_equal
AND = mybir.AluOpType.bitwise_and
SHR = mybir.AluOpType.logical_shift_right
COPY = mybir.ActivationFunctionType.Copy


CROWS = BP * L + BP * L // 4
TROWS = CROWS + 8


def _build_program():
    nc = bacc.Bacc(None, target_bir_lowering=False)
    xin_d = nc.declare_dram_parameter("xin", [TROWS, D], U8, isOutput=False)
    yin_d = nc.declare_dram_parameter("yin", [TROWS, D], U8, isOutput=False)
    out_d = nc.declare_dram_parameter("out", [BP, 1], F32, isOutput=True)
    A_d = nc.dram_tensor("A_scratch", [BP, LM, LM], F32)

    with tile.TileContext(nc) as tc:
        with (
            tc.tile_pool(name="const", bufs=1) as cpool,
            tc.tile_pool(name="ps", bufs=2, space="PSUM") as pspool,
            tc.tile_pool(name="ev", bufs=3) as evpool,
            tc.tile_pool(name="pde", bufs=1) as upool,
            tc.tile_pool(name="ablk", bufs=2) as apool,
            tc.tile_pool(name="tmp", bufs=2) as tpool,
        ):
            def load_unpack(in_d, col, tag):
                lo_d = in_d[0 : BP * L, :]
                hp_d = in_d[BP * L : CROWS, :]
                scl = cpool.tile([128, 4], U8, name=f"scl_{tag}")
                nc.gpsimd.dma_start(
                    out=scl[:],
                    in_=in_d[CROWS:TROWS, :].rearrange(
                        "a (b c) -> (a b) c", b=16, c=4
                    ),
                )
                loq = cpool.tile([128, XF], U8, name=f"loq_{tag}")
                hpq = cpool.tile([128, XF // 4], U8, name=f"hpq_{tag}")
                nc.gpsimd.dma_start(
                    out=loq[:].rearrange("q (p c d) -> q p c d", p=BP, c=2),
                    in_=lo_d.rearrange("(p c q) d -> q p c d", p=BP, c=2),
                )
                nc.gpsimd.dma_start(
                    out=hpq[:].rearrange("q (p c d) -> q p c d", p=BP // 4, c=2),
                    in_=hp_d.rearrange("(p c q) d -> q p c d", p=BP // 4, c=2),
                )
                Q = XF // 4
                hf = cpool.tile([128, XF], U8, name=f"hf_{tag}")
                sh = cpool.tile([128, Q], U8, name=f"sh_{tag}")
                nc.vector.tensor_scalar(hf[:, 0:Q], hpq[:], 3, None, AND)
                nc.vector.tensor_scalar(sh[:], hpq[:], 2, None, SHR)
                nc.vector.tensor_scalar(hf[:, Q : 2 * Q], sh[:], 3, None, AND)
                nc.vector.tensor_scalar(sh[:], hpq[:], 4, None, SHR)
                nc.vector.tensor_scalar(hf[:, 2 * Q : 3 * Q], sh[:], 3, None, AND)
                nc.vector.tensor_scalar(hf[:, 3 * Q : 4 * Q], hpq[:], 6, None, SHR)
                ht = cpool.tile([128, XF], I16, name=f"ht_{tag}")
                nc.vector.tensor_scalar(ht[:], hf[:], 256, 16384, MULT, ADD)
                xb = cpool.tile([128, XF], I16, name=f"xb_{tag}")
                nc.vector.tensor_tensor(xb[:], ht[:], loq[:], ADD)
                xq = cpool.tile([128, XF], F16, name=f"xq_{tag}")
                nc.vector.tensor_scalar(
                    xq[:], xb[:].bitcast(F16), -3.0, scl[:].bitcast(F32),
                    ADD, MULT,
                )
                return xq

            xq = load_unpack(xin_d, 0, "x")
            yq = load_unpack(yin_d, 1, "y")

            def x_ap(p, c):
                o = (p * 2 + c) * D
                return xq[:, o : o + D]

            def y_ap(p, c):
                o = (p * 2 + c) * D
                return yq[:, o : o + D]

            iot = cpool.tile([128, 2 * LM], I16)
            m1 = cpool.tile([128, 2 * LM], F16)
            m0 = cpool.tile([128, 2 * LM], F16)
            dTq = cpool.tile([128, 2 * LM], F16)
            nc.gpsimd.iota(iot[:], [[128, 2], [-1, LM]], base=0, channel_multiplier=1)
            nc.vector.tensor_scalar(m1[:], iot[:], 1, None, EQ)
            nc.vector.tensor_scalar(m0[:], iot[:], 0, None, EQ)
            nc.vector.tensor_sub(dTq[:], m1[:], m0[:])

            def dT_ap(c):
                return dTq[:, c * LM : (c + 1) * LM]

            for p in range(BP):
                dxT_ps = pspool.tile([D, LM], F32, tag="dxps", name="dxT_ps")
                dyT_ps = pspool.tile([D, LM], F32, tag="dyps", name="dyT_ps")
                for c in range(2):
                    nc.tensor.matmul(
                        dxT_ps[:], x_ap(p, c), dT_ap(c),
                        start=(c == 0), stop=(c == 1),
                    )
                for c in range(2):
                    nc.tensor.matmul(
                        dyT_ps[:], y_ap(p, c), dT_ap(c),
                        start=(c == 0), stop=(c == 1),
                    )
                dxT_sb = evpool.tile([D, LM], F32, tag="dxe", name="dxT_sb")
                dyT_sb = evpool.tile([D, LM], F32, tag="dye", name="dyT_sb")
                nc.scalar.activation(dxT_sb[:], dxT_ps[:], COPY, scale=0.5)
                nc.scalar.activation(dyT_sb[:], dyT_ps[:], COPY, scale=0.5)
                for m0_, m1_ in ((0, 128), (128, LM)):
                    a_ps = pspool.tile([128, LM], F32, tag="aps", name="a_ps")
                    nc.tensor.matmul(
                        a_ps[: m1_ - m0_, :], dxT_sb[:, m0_:m1_], dyT_sb[:],
                        start=True, stop=True,
                    )
                    a_sb = evpool.tile([128, LM], F32, tag="aev", name="a_sb", bufs=64)
                    nc.scalar.activation(
                        a_sb[: m1_ - m0_, :], a_ps[: m1_ - m0_, :], COPY, bias=-1.0
                    )
                    nc.sync.dma_start(out=A_d[p][m0_:m1_, :], in_=a_sb[: m1_ - m0_, :])

            u_bufs = [
                upool.tile([BP, N2 + 1], F32, tag=f"u{i}", name=f"u{i}")
                for i in range(2)
            ]
            nc.vector.memset(u_bufs[0][:], 1.0)
            nc.vector.memset(u_bufs[1][:], 1.0)
            step = 0
            for b in range(NBLK):
                ablk = apool.tile([BP, BLK * LM], F32, tag="ablk", name="ablk")
                nc.sync.dma_start(
                    out=ablk[:],
                    in_=A_d[:, b * BLK : (b + 1) * BLK, :].rearrange(
                        "p r a -> p (r a)"
                    ),
                )
                for r in range(BLK):
                    base = ablk[:, r * LM : (r + 1) * LM]
                    dbl = bass.AP(
                        base.tensor,
                        base.offset,
                        [base.ap[0], [base.ap[1][0], LM], [0, 2]],
                    )
                    for _ in range(2):
                        up = u_bufs[step % 2]
                        un = u_bufs[(step + 1) % 2]
                        tmp = tpool.tile([BP, N2], F32, tag="tmp", name="tmp")
                        nc.gpsimd.tensor_mul(tmp[:], up[:, 0:N2], dbl)
                        nc.vector.tensor_tensor_scan(
                            un[:, 1 : N2 + 1], up[:, 1 : N2 + 1], tmp[:],
                            1.0, ADD, ADD,
                        )
                        step += 1
            nc.sync.dma_start(out=out_d[:], in_=u_bufs[step % 2][:, N2 : N2 + 1])
    nc.compile()
    return nc


@functools.lru_cache(maxsize=1)
def _program():
    return _build_program()


@functools.lru_cache(maxsize=1)
def _executor():
    import jax
    from jax.sharding import Mesh, PartitionSpec
    from jax.experimental.shard_map import shard_map
    from concourse import bass2jax
    from concourse.bass2jax import _bass_exec_p, install_neuronx_cc_hook

    nc = _program()
    install_neuronx_cc_hook()
    partition_name = (
        nc.partition_id_tensor.name if nc.partition_id_tensor is not None else None
    )
    in_names: list[str] = []
    out_names: list[str] = []
    out_avals = []
    zero_specs = []
    for alloc in nc.m.functions[0].allocations:
        if not isinstance(alloc, mybir.MemoryLocationSet):
            continue
        name = alloc.memorylocations[0].name
        if alloc.kind == "ExternalInput":
            if name != partition_name:
                in_names.append(name)
        elif alloc.kind == "ExternalOutput":
            shape = tuple(alloc.tensor_shape)
            dtype = mybir.dt.np(alloc.dtype)
            out_names.append(name)
            out_avals.append(jax.core.ShapedArray(shape, dtype))
            zero_specs.append((shape, dtype))
    n_params = len(in_names)
    n_outs = len(out_avals)
    in_names_all = in_names + out_names + (
        [partition_name] if partition_name else []
    )
    donate = tuple(range(n_params, n_params + n_outs))

    def _body(*args):
        operands = list(args)
        if partition_name is not None:
            operands.append(bass2jax.partition_id_tensor())
        outs = _bass_exec_p.bind(
            *operands,
            out_avals=tuple(out_avals),
            in_names=tuple(in_names_all),
            out_names=tuple(out_names),
            lowering_input_output_aliases=(),
            sim_require_finite=True,
            sim_require_nnan=True,
            nc=nc,
        )
        return tuple(outs)

    devices = jax.devices()[:NCORES]
    assert len(devices) == NCORES
    mesh = Mesh(np.asarray(devices), ("core",))
    in_specs = (PartitionSpec("core"),) * (n_params + n_outs)
    out_specs = (PartitionSpec("core"),) * len(out_names)
    sharded = jax.jit(
        shard_map(
            _body, mesh=mesh, in_specs=in_specs, out_specs=out_specs,
            check_rep=False,
        ),
        donate_argnums=donate,
        keep_unused=True,
    )
    return sharded, in_names, out_names, zero_specs


try:
    import torch as _torch

    _torch.set_num_threads(4)
    _C3 = _torch.tensor(3.0, dtype=_torch.float32)
    _C1 = _torch.tensor(1.0, dtype=_torch.float32)
except Exception:
    _torch = None

_C_SRC = r"""
#include <stdint.h>
#include <math.h>

float absmax_f32(const float* x, long n) {
    float m = 0.f;
    for (long i = 0; i < n; i++) {
        float v = fabsf(x[i]);
        if (v > m) m = v;
    }
    return m;
}

/* x: [ncores][32][256*64] f32 -> out: per core 32*PL lo bytes then 8*PL
   packed-hi bytes (core-major, so axis-0 sharding slices stay contiguous).
   10-bit value m = round(x*k + 512) in [0,1023]; lo = m & 255; hp packs the
   hi-2-bits of pairs (pb, pb+8, pb+16, pb+24) into one byte. */
void quant10(const float* x, uint8_t* out, float k, long ncores,
             long core_stride) {
    const long PL = 256 * 64;
    for (long c = 0; c < ncores; c++) {
        const float* xc = x + c * 32 * PL;
        uint8_t* loc = out + c * core_stride;
        uint8_t* hpc = loc + 32 * PL;
        for (long pb = 0; pb < 8; pb++) {
            const float* x0 = xc + pb * PL;
            const float* x1 = x0 + 8 * PL;
            const float* x2 = x0 + 16 * PL;
            const float* x3 = x0 + 24 * PL;
            uint8_t* l0 = loc + pb * PL;
            uint8_t* l1 = l0 + 8 * PL;
            uint8_t* l2 = l0 + 16 * PL;
            uint8_t* l3 = l0 + 24 * PL;
            uint8_t* h = hpc + pb * PL;
            for (long i = 0; i < PL; i++) {
                int m0 = (int)(x0[i] * k + 512.5f);
                int m1 = (int)(x1[i] * k + 512.5f);
                int m2 = (int)(x2[i] * k + 512.5f);
                int m3 = (int)(x3[i] * k + 512.5f);
                m0 = m0 < 0 ? 0 : (m0 > 1023 ? 1023 : m0);
                m1 = m1 < 0 ? 0 : (m1 > 1023 ? 1023 : m1);
                m2 = m2 < 0 ? 0 : (m2 > 1023 ? 1023 : m2);
                m3 = m3 < 0 ? 0 : (m3 > 1023 ? 1023 : m3);
                l0[i] = (uint8_t)(m0 & 255);
                l1[i] = (uint8_t)(m1 & 255);
                l2[i] = (uint8_t)(m2 & 255);
                l3[i] = (uint8_t)(m3 & 255);
                h[i] = (uint8_t)((m0 >> 8) | ((m1 >> 8) << 2)
                                | ((m2 >> 8) << 4) | ((m3 >> 8) << 6));
            }
        }
    }
}
"""


def _build_cquant():
    import ctypes
    import hashlib
    import os
    import subprocess

    h = hashlib.md5(_C_SRC.encode()).hexdigest()[:12]
    so = f"/tmp/_sigq_{h}.so"
    if not os.path.exists(so):
        cpath = f"/tmp/_sigq_{h}.c"
        with open(cpath, "w") as f:
            f.write(_C_SRC)
        tmp = so + f".{os.getpid()}.tmp"
        subprocess.run(
            ["gcc", "-O3", "-march=native", "-funroll-loops", "-shared",
             "-fPIC", cpath, "-o", tmp, "-lm"],
            check=True, capture_output=True,
        )
        os.replace(tmp, so)
    lib = ctypes.CDLL(so)
    lib.absmax_f32.restype = ctypes.c_float
    lib.absmax_f32.argtypes = [ctypes.c_void_p, ctypes.c_long]
    lib.quant10.restype = None
    lib.quant10.argtypes = [
        ctypes.c_void_p, ctypes.c_void_p, ctypes.c_float, ctypes.c_long,
        ctypes.c_long,
    ]
    return lib


try:
    _clib = _build_cquant()
except Exception:
    _clib = None


def _scale_rows(s: float) -> np.ndarray:
    return np.full((128,), s, np.float32).view(np.uint8).reshape(8, D)


def _quant10_c(arr: np.ndarray):
    a = np.ascontiguousarray(arr, np.float32).reshape(-1)
    s = float(_clib.absmax_f32(a.ctypes.data, a.size)) * 1.002
    if s == 0.0:
        s = 1.0
    comb = np.empty(NCORES * TROWS * D, np.uint8)
    _clib.quant10(a.ctypes.data, comb.ctypes.data,
                  np.float32(512.0 / s), NCORES, TROWS * D)
    v = comb.reshape(NCORES, TROWS, D)
    v[:, CROWS:TROWS] = _scale_rows(s)
    return comb.reshape(NCORES * TROWS, D)


def _pack_fallback(arr: np.ndarray):
    lo, bits, s = _quant10_lo(arr)
    hp = _quant10_hp(bits)
    comb = np.empty((NCORES, TROWS, D), np.uint8)
    comb[:, 0 : BP * L] = lo.reshape(NCORES, BP * L, D)
    comb[:, BP * L : CROWS] = hp.reshape(NCORES, BP * L // 4, D)
    comb[:, CROWS:TROWS] = _scale_rows(s)
    return comb.reshape(NCORES * TROWS, D)


def _quant10_lo(arr: np.ndarray):
    if _torch is not None:
        t = _torch.from_numpy(np.ascontiguousarray(arr, np.float32)).view(-1)
        mn, mx = _torch.aminmax(t)
        s = float(max(-mn.item(), mx.item())) * 1.002
        if s == 0.0:
            s = 1.0
        z = _torch.addcmul(_C3, t, _C1, value=1.0 / s).to(_torch.float16)
        bits = z.view(_torch.int16)
        lo = bits.to(_torch.uint8)
        return lo.numpy().reshape(B * L, D), bits, s
    a = np.ascontiguousarray(arr, np.float32).reshape(-1)
    s = float(np.abs(a).max()) * 1.002
    if s == 0.0:
        s = 1.0
    bits = (a / s + 3.0).astype(np.float16).view(np.int16)
    return (bits & 255).astype(np.uint8).reshape(B * L, D), bits, s


def _quant10_hp(bits):
    if _torch is not None:
        hi = (bits >> 8) & 3
        v = hi.view(NCORES, BP, L * D)
        hp = (
            v[:, 0 : BP // 4]
            | (v[:, BP // 4 : BP // 2] << 2)
            | (v[:, BP // 2 : 3 * BP // 4] << 4)
            | (v[:, 3 * BP // 4 : BP] << 6)
        ).to(_torch.uint8)
        return hp.numpy().reshape(B * L // 4, D)
    v = ((bits >> 8) & 3).reshape(NCORES, BP, L * D)
    hp = (
        v[:, 0 : BP // 4]
        | (v[:, BP // 4 : BP // 2] << 2)
        | (v[:, BP // 2 : 3 * BP // 4] << 4)
        | (v[:, 3 * BP // 4 : BP] << 6)
    ).astype(np.uint8)
    return hp.reshape(B * L // 4, D)


@functools.lru_cache(maxsize=1)
def _sharding():
    import jax
    from jax.sharding import Mesh, NamedSharding, PartitionSpec

    mesh = Mesh(np.asarray(jax.devices()[:NCORES]), ("core",))
    return NamedSharding(mesh, PartitionSpec("core"))


def kernel(xs: np.ndarray, ys: np.ndarray) -> np.ndarray:
    import jax

    sharded, in_names, out_names, zero_specs = _executor()
    sh = _sharding()
    quant = _quant10_c if _clib is not None else _pack_fallback
    feeds = {"xin": jax.device_put(quant(np.asarray(xs)), sh)}
    feeds["yin"] = jax.device_put(quant(np.asarray(ys)), sh)
    concat_in = [feeds[name] for name in in_names]
    concat_zeros = [
        np.zeros((NCORES * s[0], *s[1:]), dt) for s, dt in zero_specs
    ]
    out_arrs = sharded(*concat_in, *concat_zeros)
    out = np.asarray(out_arrs[out_names.index("out")])
    return out.reshape(B).astype(np.float32, copy=False)
